# revision 1
# baseline (speedup 1.0000x reference)
"""Trainium2 Bass kernel for gnn_message_passing (nn_BFR_28089086116615).

Sharding: receiver axis i (G=4096 -> 8 cores x 512). Host pre-transposes the
edge matrices and folds the {coef, 1} gate weights in bf16: wT[j, i]. On
device, sigma^T is computed natively in [j-partition, i-free] layout (ACT
sigmoid, per-partition bias = s_src[j-chunk], input = broadcast s_dst row),
gated by wT on DVE (bf16 2x), and contracted on PE with stationary weights
[1 | h] so the receiver rowsum lands in psum row 0. s_src comes from a DVE
multiply+reduce over the natural-layout h (no PE involvement). BatchNorm is
per-gene -> fully local; two per-batch AllGathers of normalized h between the
blocks so block-2 can start on batch 0 while batch 1 is still in flight.
"""
import sys
sys.path.insert(0, "/opt/trn_rl_repo")
import numpy as np
import ml_dtypes

import concourse.bass as bass
import concourse.bacc as bacc
import concourse.mybir as mybir
import concourse.tile as tile
from concourse.bass_utils import run_bass_kernel_spmd

NC = 8
B, G, NI, H, NO = 2, 4096, 8, 32, 32
GL = G // NC              # 512 local receivers per core
LCH = GL // 128           # 4 local chunks
NCH = G // 128            # 32 global j-chunks
QC = 8                    # j-chunks per sigma quarter-slab
W1 = H + 1                # group width: [1 | h]
ALPHA, BETA, BN_EPS = 0.005, 5e-5, 1e-5

F32 = mybir.dt.float32
BF16 = mybir.dt.bfloat16
AF = mybir.ActivationFunctionType
ALU = mybir.AluOpType
XY = mybir.AxisListType.XY
AX = mybir.AxisListType.X

_CACHE = {}

# Prefer table sets so {Exp, Ln, Square} share one set: 5 loads total.
_orig_tables = None


def _patched_tables(arch):
    tabs = _orig_tables(arch)
    order = ["natural_log_exp_and_others", "sigmoid_and_others"]
    out = {k: tabs[k] for k in order if k in tabs}
    out.update({k: v for k, v in tabs.items() if k not in out})
    return out


def build_program():

    nc = bacc.Bacc("TRN2", target_bir_lowering=False, debug=False,
                   enable_asserts=False, num_devices=NC)

    def din(name, shape, dt):
        return nc.dram_tensor(name, shape, dt, kind="ExternalInput").ap()

    xT_aug = din("xT_aug", [NI + 1, B * G], F32)           # row 8 = ones
    xT_loc = din("xT_loc", [NI + 1, B * GL], F32)          # row 8 = ones
    w1T = din("w1T", [G, GL], BF16)
    w2T = din("w2T", [G, GL], BF16)
    W_aug = din("W_aug", [NI + 1, H], F32)
    We1_f = din("We1_f", [H + 1, 2], F32)
    We2_f = din("We2_f", [H + 1, 2], F32)
    We1_rep = din("We1_rep", [1, NCH * H], BF16)
    We2_rep = din("We2_rep", [1, NCH * H], BF16)
    Wn1a = din("Wn1a", [H + 1, NO], F32)                   # [0; W_n[:H]]
    Wn1b = din("Wn1b", [H + 1, NO], F32)                   # [W_n[H:]; b_n]
    Wm1a = din("Wm1a", [H + 1, NO], F32)
    Wm1b = din("Wm1b", [H + 1, NO], F32)
    Wn2a = din("Wn2a", [H + 1, NO], F32)
    Wn2b = din("Wn2b", [H + 1, NO], F32)
    Wm2a = din("Wm2a", [H + 1, NO], F32)
    Wm2b = din("Wm2b", [H + 1, NO], F32)
    bn_g_nat = din("bn_g_nat", [128, LCH], F32)
    bn_b_nat = din("bn_b_nat", [128, LCH], F32)
    bn_g_row = din("bn_g_row", [1, GL], F32)
    bn_b_row = din("bn_b_row", [1, GL], F32)

    out = nc.dram_tensor("out", [B * GL, NO], F32, kind="ExternalOutput").ap()
    out_r = out.rearrange("(b l p) f -> p b l f", b=B, l=LCH, p=128)

    with tile.TileContext(nc) as tc:
        with (
            tc.tile_pool(name="cp", bufs=1) as cp,
            tc.tile_pool(name="bp", bufs=1) as bp,
            tc.tile_pool(name="wp", bufs=1) as wp,
            tc.tile_pool(name="sp", bufs=2) as sp,
            tc.tile_pool(name="pp", bufs=1, space="PSUM") as pp,
            tc.tile_pool(name="dp", bufs=1, space="DRAM") as dp,
        ):
            # ---------- constants (small DMAs first: they gate compute) ----
            W_aug_sb = cp.tile([NI + 1, H], F32, name="W_aug_sb", tag="W_aug_sb")
            nc.sync.dma_start(W_aug_sb[:], W_aug[:])
            sm = {}
            for nm, ap_ in [("We1_rep", We1_rep), ("We2_rep", We2_rep),
                            ("We1_f", We1_f), ("We2_f", We2_f),
                            ("Wn1a", Wn1a), ("Wn1b", Wn1b),
                            ("Wm1a", Wm1a), ("Wm1b", Wm1b),
                            ("Wn2a", Wn2a), ("Wn2b", Wn2b),
                            ("Wm2a", Wm2a), ("Wm2b", Wm2b),
                            ("bn_g_nat", bn_g_nat), ("bn_b_nat", bn_b_nat),
                            ("bn_g_row", bn_g_row), ("bn_b_row", bn_b_row)]:
                t = cp.tile(list(ap_.shape), ap_.dtype, name=f"{nm}_sb",
                            tag=f"{nm}_sb")
                nc.sync.dma_start(t[:], ap_[:])
                sm[nm] = t
            ones_c = cp.tile([1, 128], F32, name="ones_c", tag="ones_c")
            nc.vector.memset(ones_c[:], 1.0)
            ones_cb = cp.tile([1, 128], BF16, name="ones_cb", tag="ones_cb")
            nc.vector.memset(ones_cb[:], 1.0)
            onesk = cp.tile([H, 1], F32, name="onesk", tag="onesk")
            nc.vector.memset(onesk[:], 1.0)
            xTl_sb = cp.tile([NI + 1, B * GL], F32, name="xTl_sb", tag="xTl_sb")
            nc.sync.dma_start(xTl_sb[:], xT_loc[:])

            # ---------- big resident tensors ----------
            h0n = bp.tile([128, B * NCH * W1], BF16, name="h0n", tag="h0n")
            h0l = bp.tile([H + 1, B * GL], F32, name="h0l", tag="h0l")
            nodes1T = bp.tile([H + 1, B * GL], F32, name="nodes1T", tag="nodes1T")
            nodes2T = bp.tile([H + 1, B * GL], F32, name="nodes2T", tag="nodes2T")
            hbnT_f = bp.tile([H + 1, B * GL], F32, name="hbnT_f", tag="hbnT_f")
            ghat = [bp.tile([128, NC * LCH * W1], BF16, name=f"ghat{b}",
                            tag=f"ghat{b}") for b in range(B)]
            nc.vector.memset(h0n[:], 1.0)
            nc.vector.memset(h0l[H:H + 1, :], 1.0)
            nc.vector.memset(nodes1T[H:H + 1, :], 1.0)
            nc.vector.memset(nodes2T[H:H + 1, :], 1.0)
            nc.vector.memset(hbnT_f[H:H + 1, :], 1.0)

            def elu(z_psum, out_ap, shape):
                p, f = shape
                tf = wp.tile([128, GL], F32, name="elu_t", tag="elu_t", bufs=3)
                t1 = tf[0:p, 0:f]
                nc.vector.tensor_scalar_min(t1, z_psum, 0.0)
                nc.scalar.activation(t1, t1, AF.Exp)
                nc.vector.tensor_scalar_add(t1, t1, -1.0)
                nc.vector.tensor_tensor(out_ap, z_psum, t1, op=ALU.max)

            # ---------- phase 1: h0 (natural layout, groups [1|h]) ----------
            h0n_v = h0n.rearrange("p (q e) -> p q e", e=W1)
            for kq in range(8):
                xq = wp.tile([NI + 1, 8 * 128], F32, name="xq", tag="xq", bufs=2)
                nc.sync.dma_start(xq[:], xT_aug[:, kq * 1024:(kq + 1) * 1024])
                ps = pp.tile([128, 8 * H], F32, name="ps_sm", tag="sm", bufs=4)
                for s in range(8):
                    nc.tensor.matmul(ps[:, s * H:(s + 1) * H],
                                     xq[:, s * 128:(s + 1) * 128],
                                     W_aug_sb[:], start=True, stop=True)
                elu(ps[:], h0n_v[:, kq * 8:(kq + 1) * 8, 1:W1], [128, 8 * H])
            for b in range(B):
                ps = pp.tile([H, GL], F32, name="ps_sm", tag="sm", bufs=4)
                nc.tensor.matmul(ps[:], W_aug_sb[:],
                                 xTl_sb[:, b * GL:(b + 1) * GL],
                                 start=True, stop=True)
                elu(ps[:], h0l[0:H, b * GL:(b + 1) * GL], [H, GL])

            # big edge-weight DMAs issued after the gating small ones
            w1T_sb = bp.tile([128, NCH * GL], BF16, name="w1T_sb", tag="w1T_sb")
            w2T_sb = bp.tile([128, NCH * GL], BF16, name="w2T_sb", tag="w2T_sb")
            w1T_r = w1T.rearrange("(k p) i -> p k i", p=128)
            w2T_r = w2T.rearrange("(k p) i -> p k i", p=128)
            for kq in range(4):
                nc.sync.dma_start(
                    w1T_sb[:, kq * QC * GL:(kq + 1) * QC * GL],
                    w1T_r[:, kq * QC:(kq + 1) * QC])

            gather_in = dp.tile([128, B * LCH * W1], BF16, name="gin",
                                tag="gin")
            gather_out = dp.tile([NC * 128, B * LCH * W1], BF16,
                                 addr_space="Shared", name="gout", tag="gout")

            # ---------- one message-passing block ----------
            def mp_block(blk, wT_sb, We_rep, We_f, Wna, Wnb, Wma, Wmb,
                         nat_of, hTl, nodesT, merge_dst):
                # s_src[p, col] = sum_f h_nat[p, g*33+1+f] * We_src[f]  (DVE)
                wrep = wp.tile([128, NCH * H], BF16, name="wrep", tag="wrep",
                               bufs=1)
                for c4 in range(NCH * H // 512):
                    ps_w = pp.tile([128, 512], F32, name="ps_w", tag="bc",
                                   bufs=2)
                    nc.tensor.matmul(ps_w[:], ones_cb[:],
                                     We_rep[:, c4 * 512:(c4 + 1) * 512],
                                     start=True, stop=True)
                    nc.vector.tensor_copy(wrep[:, c4 * 512:(c4 + 1) * 512],
                                          ps_w[:])
                wrep_v = wrep.rearrange("p (q f) -> p q f", f=H)
                ssrc = wp.tile([128, B * NCH], F32, name=f"ssrc{blk}",
                               tag=f"ssrc{blk}")
                for b in range(B):
                    h_nat, goff = nat_of(b)
                    h_nat_v = h_nat.rearrange("p (q e) -> p q e", e=W1)
                    ssx = wp.tile([128, NCH * H], BF16, name="ssx", tag="ssx",
                                  bufs=2)
                    ssx_v = ssx.rearrange("p (q f) -> p q f", f=H)
                    nc.vector.tensor_tensor(
                        ssx_v, h_nat_v[:, goff:goff + NCH, 1:W1], wrep_v,
                        op=ALU.mult)
                    nc.vector.reduce_sum(ssrc[:, b * NCH:(b + 1) * NCH],
                                         ssx_v, axis=AX)
                accs = []
                for b in range(B):
                    h_nat, goff = nat_of(b)
                    h_nat_v = h_nat.rearrange("p (q e) -> p q e", e=W1)
                    ps_d = pp.tile([1, GL], F32, name="ps_d", tag="sm", bufs=4)
                    nc.tensor.matmul(ps_d[:], We_f[:, 1:2],
                                     hTl[:, b * GL:(b + 1) * GL],
                                     start=True, stop=True)
                    sd_row = wp.tile([1, GL], F32, name="sd_row", tag="sd_row",
                                     bufs=2)
                    nc.vector.tensor_copy(sd_row[:], ps_d[:])
                    ps_bc = pp.tile([128, GL], F32, name="ps_bc", tag="bc",
                                    bufs=2)
                    nc.tensor.matmul(ps_bc[:], ones_c[:], sd_row[:],
                                     start=True, stop=True)
                    sdb = wp.tile([128, GL], F32, name="sdb", tag="sdb", bufs=2)
                    nc.vector.tensor_copy(sdb[:], ps_bc[:])

                    ps_acc = pp.tile([W1, GL], F32, name="ps_acc", tag="acc",
                                     bufs=2)
                    for qq in range(NCH // QC):
                        sig = sp.tile([128, QC * GL], BF16, name="sig",
                                      tag="sig", bufs=2)
                        for k8 in range(QC):
                            k = qq * QC + k8
                            nc.scalar.activation(
                                sig[:, k8 * GL:(k8 + 1) * GL], sdb[:],
                                AF.Sigmoid,
                                bias=ssrc[:, b * NCH + k:b * NCH + k + 1])
                        for hh in range(QC // 4):
                            sl = slice(hh * 4 * GL, (hh + 1) * 4 * GL)
                            wsl = slice((qq * QC + hh * 4) * GL,
                                        (qq * QC + hh * 4 + 4) * GL)
                            nc.vector.tensor_tensor(sig[:, sl], sig[:, sl],
                                                    wT_sb[:, wsl], op=ALU.mult)
                        for k8 in range(QC):
                            k = qq * QC + k8
                            nc.tensor.matmul(
                                ps_acc[:], h_nat_v[:, goff + k, :],
                                sig[:, k8 * GL:(k8 + 1) * GL],
                                start=(k == 0), stop=(k == NCH - 1))
                    accs.append(ps_acc)
                for b in range(B):
                    ps_acc = accs[b]
                    # rows: 0 = rowsum, 1..32 = recv_srcT
                    rfull = wp.tile([H + 1, GL], F32, name="rfull", tag="rfull",
                                    bufs=2)
                    nc.vector.tensor_copy(rfull[:], ps_acc[:])
                    ps_rb = pp.tile([H, GL], F32, name="ps_rb", tag="bc", bufs=2)
                    nc.tensor.matmul(ps_rb[:], ones_c[:, 0:H], rfull[0:1, :],
                                     start=True, stop=True)
                    hdT = wp.tile([H + 1, GL], F32, name="hdT", tag="hdT",
                                  bufs=2)
                    nc.vector.tensor_tensor(hdT[0:H, :],
                                            hTl[0:H, b * GL:(b + 1) * GL],
                                            ps_rb[:], op=ALU.mult)
                    nc.vector.memset(hdT[H:H + 1, :], 1.0)
                    ps_n = pp.tile([H, GL], F32, name="ps_n", tag="sm", bufs=4)
                    nc.tensor.matmul(ps_n[:], Wna[:], rfull[:],
                                     start=True, stop=False)
                    nc.tensor.matmul(ps_n[:], Wnb[:], hdT[:],
                                     start=False, stop=True)
                    elu(ps_n[:], nodesT[0:H, b * GL:(b + 1) * GL], [H, GL])
                    ps_m = pp.tile([128, LCH * NO], F32, name="ps_m", tag="sm",
                                   bufs=4)
                    for l in range(LCH):
                        c0 = b * GL + l * 128
                        nc.tensor.matmul(ps_m[:, l * NO:(l + 1) * NO],
                                         nodesT[:, c0:c0 + 128],
                                         Wma[:], start=True, stop=False)
                        nc.tensor.matmul(ps_m[:, l * NO:(l + 1) * NO],
                                         hTl[:, c0:c0 + 128],
                                         Wmb[:], start=False, stop=True)
                    merge_dst(b, ps_m)

            # ---------- block 1 ----------
            h1n = wp.tile([128, B * LCH * NO], F32, name="h1n", tag="h1n")

            def merge1_dst(b, ps_m):
                c0 = b * LCH * NO
                elu(ps_m[:], h1n[:, c0:c0 + LCH * NO], [128, LCH * NO])

            mp_block(1, w1T_sb, sm["We1_rep"], sm["We1_f"],
                     sm["Wn1a"], sm["Wn1b"], sm["Wm1a"], sm["Wm1b"],
                     lambda b: (h0n, b * NCH), h0l, nodes1T, merge1_dst)
            for kq in range(4):
                nc.sync.dma_start(
                    w2T_sb[:, kq * QC * GL:(kq + 1) * QC * GL],
                    w2T_r[:, kq * QC:(kq + 1) * QC])

            h1T = wp.tile([H, B * GL], F32, name="h1T", tag="h1T")
            for b in range(B):
                ps = pp.tile([H, GL], F32, name="ps_sm2", tag="sm", bufs=4)
                nc.tensor.matmul(ps[:], sm["Wm1a"][:],
                                 nodes1T[:, b * GL:(b + 1) * GL],
                                 start=True, stop=False)
                nc.tensor.matmul(ps[:], sm["Wm1b"][:],
                                 h0l[:, b * GL:(b + 1) * GL],
                                 start=False, stop=True)
                elu(ps[:], h1T[:, b * GL:(b + 1) * GL], [H, GL])

            # ---------- BatchNorm (fully local) ----------
            stat = wp.tile([128, 6 * LCH], F32, name="stat", tag="stat")
            mu_n, var_n = stat[:, 0:LCH], stat[:, LCH:2 * LCH]
            scl_n, shf_n = stat[:, 2 * LCH:3 * LCH], stat[:, 3 * LCH:4 * LCH]
            t_n, t2_n = stat[:, 4 * LCH:5 * LCH], stat[:, 5 * LCH:6 * LCH]
            sq_n = wp.tile([128, B * LCH * NO], F32, name="sq_n", tag="sq_n")
            nc.scalar.activation(sq_n[:], h1n[:], AF.Square)
            h1n_r = h1n.rearrange("p (b l f) -> p b l f", b=B, l=LCH)
            sq_r = sq_n.rearrange("p (b l f) -> p b l f", b=B, l=LCH)
            for l in range(LCH):
                nc.vector.reduce_sum(mu_n[:, l:l + 1], h1n_r[:, :, l, :], axis=XY)
                nc.vector.reduce_sum(var_n[:, l:l + 1], sq_r[:, :, l, :], axis=XY)
            nc.vector.tensor_scalar_mul(mu_n, mu_n, 1.0 / (B * NO))
            nc.vector.tensor_scalar_mul(var_n, var_n, 1.0 / (B * NO))
            nc.vector.tensor_tensor(t_n, mu_n, mu_n, op=ALU.mult)
            nc.vector.tensor_tensor(var_n, var_n, t_n, op=ALU.subtract)
            nc.vector.tensor_scalar_add(t_n, var_n, BN_EPS)
            nc.scalar.activation(t_n, t_n, AF.Ln)
            nc.scalar.activation(t_n, t_n, AF.Exp, scale=-0.5)
            nc.vector.tensor_tensor(scl_n, t_n, sm["bn_g_nat"][:], op=ALU.mult)
            nc.vector.tensor_tensor(t2_n, mu_n, scl_n, op=ALU.mult)
            nc.vector.tensor_tensor(shf_n, sm["bn_b_nat"][:], t2_n,
                                    op=ALU.subtract)
            # normalized h, natural groups [1|h]; per-b gather as soon as ready
            hbn_n = wp.tile([128, B * LCH * W1], BF16, name="hbn_n",
                            tag="hbn_n")
            nc.vector.memset(hbn_n[:], 1.0)
            for b in range(B):
                for l in range(LCH):
                    q = b * LCH + l
                    nc.vector.tensor_scalar(
                        hbn_n[:, q * W1 + 1:(q + 1) * W1],
                        h1n[:, (b * LCH + l) * NO:(b * LCH + l + 1) * NO],
                        scl_n[:, l:l + 1], shf_n[:, l:l + 1],
                        op0=ALU.mult, op1=ALU.add)
            nc.sync.dma_start(gather_in[:], hbn_n[:])
            nc.gpsimd.collective_compute(
                "AllGather", ALU.bypass, replica_groups=[list(range(NC))],
                ins=[gather_in.opt()], outs=[gather_out.opt()])
            for b in range(B):
                for c in range(NC):
                    nc.sync.dma_start(
                        ghat[b][:, c * LCH * W1:(c + 1) * LCH * W1],
                        gather_out[c * 128:(c + 1) * 128,
                                   b * LCH * W1:(b + 1) * LCH * W1])

            # row-layout stats for the feature-major copy
            rowb = wp.tile([1, 4 * GL], F32, name="rowb", tag="rowb")
            mu_r, var_r = rowb[:, 0:GL], rowb[:, GL:2 * GL]
            scl_r, shf_r = rowb[:, 2 * GL:3 * GL], rowb[:, 3 * GL:4 * GL]
            t_r, t2_r = scl_r, shf_r
            sqT = wp.tile([H, B * GL], F32, name="sqT", tag="sqT")
            nc.scalar.activation(sqT[:], h1T[:], AF.Square)
            ps_r0 = pp.tile([1, GL], F32, name="ps_r0", tag="sm", bufs=4)
            for b in range(B):
                nc.tensor.matmul(ps_r0[:], onesk[:],
                                 h1T[:, b * GL:(b + 1) * GL],
                                 start=(b == 0), stop=(b == B - 1))
            ps_r1 = pp.tile([1, GL], F32, name="ps_r1", tag="sm", bufs=4)
            for b in range(B):
                nc.tensor.matmul(ps_r1[:], onesk[:],
                                 sqT[:, b * GL:(b + 1) * GL],
                                 start=(b == 0), stop=(b == B - 1))
            nc.vector.tensor_scalar_mul(mu_r, ps_r0[:], 1.0 / (B * NO))
            nc.vector.tensor_scalar_mul(var_r, ps_r1[:], 1.0 / (B * NO))
            nc.vector.tensor_tensor(t_r, mu_r, mu_r, op=ALU.mult)
            nc.vector.tensor_tensor(var_r, var_r, t_r, op=ALU.subtract)
            nc.vector.tensor_scalar_add(t_r, var_r, BN_EPS)
            nc.scalar.activation(t_r, t_r, AF.Ln)
            nc.scalar.activation(t_r, t_r, AF.Exp, scale=-0.5)
            nc.vector.tensor_tensor(scl_r, t_r, sm["bn_g_row"][:], op=ALU.mult)
            nc.vector.tensor_tensor(t2_r, mu_r, scl_r, op=ALU.mult)
            nc.vector.tensor_tensor(shf_r, sm["bn_b_row"][:], t2_r,
                                    op=ALU.subtract)
            ps_sc = pp.tile([H, GL], F32, name="ps_sc", tag="bc", bufs=2)
            nc.tensor.matmul(ps_sc[:], ones_c[:, 0:H], scl_r, start=True,
                             stop=True)
            ps_sh = pp.tile([H, GL], F32, name="ps_sh", tag="bc", bufs=2)
            nc.tensor.matmul(ps_sh[:], ones_c[:, 0:H], shf_r, start=True,
                             stop=True)
            for b in range(B):
                sl = slice(b * GL, (b + 1) * GL)
                nc.vector.tensor_tensor(hbnT_f[0:H, sl], h1T[:, sl], ps_sc[:],
                                        op=ALU.mult)
                nc.vector.tensor_tensor(hbnT_f[0:H, sl], hbnT_f[0:H, sl],
                                        ps_sh[:], op=ALU.add)

            # ---------- block 2 ----------
            out_n = wp.tile([128, B * LCH * NO], F32, name="out_n", tag="out_n")

            def merge2_dst(b, ps_m):
                c0 = b * LCH * NO
                elu(ps_m[:], out_n[:, c0:c0 + LCH * NO], [128, LCH * NO])

            mp_block(2, w2T_sb, sm["We2_rep"], sm["We2_f"],
                     sm["Wn2a"], sm["Wn2b"], sm["Wm2a"], sm["Wm2b"],
                     lambda b: (ghat[b], 0), hbnT_f, nodes2T, merge2_dst)

            nc.sync.dma_start(out_r, out_n[:])

    nc.compile()
    return nc


def _prep_inputs(x, edges1, edges2, W_infer, b_infer, W_e1, b_e1, W_e2, b_e2,
                 W_n1, b_n1, W_n2, b_n2, W_m1, b_m1, W_m2, b_m2,
                 bn_gamma, bn_beta):
    f32 = np.float32
    bf16 = ml_dtypes.bfloat16
    xT = np.asarray(x, f32).transpose(2, 0, 1).reshape(NI, B * G)
    xT_aug = np.concatenate([xT, np.ones((1, B * G), f32)], axis=0)
    w1 = (ALPHA + (1.0 - ALPHA) * np.asarray(edges1, f32)).astype(bf16)
    w2 = (BETA + (1.0 - BETA) * np.asarray(edges2, f32)).astype(bf16)

    def wecat(W_e, b_e):
        c0 = np.concatenate([np.asarray(W_e, f32)[:H, 0], [0.0]]).astype(f32)
        c1 = np.concatenate([np.asarray(W_e, f32)[H:, 0],
                             [np.asarray(b_e, f32)[0]]]).astype(f32)
        return np.stack([c0, c1], axis=1)

    We1 = wecat(W_e1, b_e1)
    We2 = wecat(W_e2, b_e2)
    z = np.zeros((1, NO), f32)

    def stk(Wpart, brow):
        return np.concatenate([np.asarray(Wpart, f32), brow], 0)

    com = dict(
        xT_aug=xT_aug,
        W_aug=np.concatenate([np.asarray(W_infer, f32),
                              np.asarray(b_infer, f32)[None, :]], 0),
        We1_rep=np.tile(We1[:H, 0], NCH)[None, :].astype(bf16),
        We2_rep=np.tile(We2[:H, 0], NCH)[None, :].astype(bf16),
        We1_f=We1, We2_f=We2,
        Wn1a=np.concatenate([z, np.asarray(W_n1, f32)[:H]], 0),
        Wn1b=stk(np.asarray(W_n1, f32)[H:], np.asarray(b_n1, f32)[None, :]),
        Wm1a=stk(np.asarray(W_m1, f32)[:H], np.asarray(b_m1, f32)[None, :]),
        Wm1b=stk(np.asarray(W_m1, f32)[H:], z),
        Wn2a=np.concatenate([z, np.asarray(W_n2, f32)[:H]], 0),
        Wn2b=stk(np.asarray(W_n2, f32)[H:], np.asarray(b_n2, f32)[None, :]),
        Wm2a=stk(np.asarray(W_m2, f32)[:H], np.asarray(b_m2, f32)[None, :]),
        Wm2b=stk(np.asarray(W_m2, f32)[H:], z),
    )
    in_maps = []
    for c in range(NC):
        sl = slice(c * GL, (c + 1) * GL)
        xl = np.asarray(x, f32)[:, sl, :].transpose(2, 0, 1).reshape(NI, B * GL)
        m = dict(com)
        m["xT_loc"] = np.concatenate([xl, np.ones((1, B * GL), f32)], 0)
        m["w1T"] = np.ascontiguousarray(w1[sl, :].T)
        m["w2T"] = np.ascontiguousarray(w2[sl, :].T)
        g = np.asarray(bn_gamma, f32)[sl]
        b_ = np.asarray(bn_beta, f32)[sl]
        m["bn_g_nat"] = np.ascontiguousarray(g.reshape(LCH, 128).T)
        m["bn_b_nat"] = np.ascontiguousarray(b_.reshape(LCH, 128).T)
        m["bn_g_row"] = np.ascontiguousarray(g[None, :])
        m["bn_b_row"] = np.ascontiguousarray(b_[None, :])
        in_maps.append(m)
    return in_maps


def kernel(**inputs):
    if "nc" not in _CACHE:
        _CACHE["nc"] = build_program()
    nc = _CACHE["nc"]
    in_maps = _prep_inputs(**inputs)
    res = run_bass_kernel_spmd(nc, in_maps, list(range(NC)))
    parts = [res.results[c]["out"].reshape(B, GL, NO) for c in range(NC)]
    return np.concatenate(parts, axis=1).astype(np.float32)



# revision 21
# speedup vs baseline: 1.2547x; 1.2547x over previous
"""Trainium2 Bass kernel for gnn_message_passing (nn_BFR_28089086116615).

Sharding: receiver axis i (G=4096 -> 8 cores x 512). Host pre-transposes the
edge matrices and folds the {coef, 1} gate weights in bf16, pre-swizzled so
each core's [j-part, chunk, i] slab is DMA-contiguous.

Per block the gate field sigma^T is produced in [j-part, i-free] layout:
DVE tensor_scalar (bf16 4x) adds the per-chunk s_src bias onto a broadcast
s_dst row tile, ACT applies sigmoid over 4-chunk groups (FD=2048) in place,
DVE multiplies the folded edge weights in (bf16 2x), and PE contracts with
stationary 34-wide groups [1 | aux | h] so rowsum lands in psum row 0.

BatchNorm is per-gene (fully local stats). Between the blocks three
AllGathers run back to back: a tiny one carrying [s_src2 | scale | shift]
(which is all the block-2 sigmoid loop needs) and one raw-h1 gather per
batch (issued right after that batch's merge) that only gates the PE
accumulation; block-2's stationary groups fold BN as [1 | shift | scale*h1]
with a host-built extra weight row, so remote normalization is a single
per-chunk scale multiply.
"""
import sys
sys.path.insert(0, "/opt/trn_rl_repo")
import numpy as np
import ml_dtypes

import concourse.bass as bass
import concourse.bacc as bacc
import concourse.mybir as mybir
import concourse.tile as tile
from concourse.bass_utils import run_bass_kernel_spmd

NC = 8
B, G, NI, H, NO = 2, 4096, 8, 32, 32
GL = G // NC              # 512 local receivers per core
LCH = GL // 128           # 4 local chunks
NCH = G // 128            # 32 global j-chunks
GRP = 4                   # j-chunks per sigma group (ACT FD = GRP*GL)
NG = NCH // GRP           # 8 groups per batch
W4G = 34                  # group width: [1 | aux | h]
ALPHA, BETA, BN_EPS = 0.005, 5e-5, 1e-5

F32 = mybir.dt.float32
BF16 = mybir.dt.bfloat16
AF = mybir.ActivationFunctionType
ALU = mybir.AluOpType
XY = mybir.AxisListType.XY
AX = mybir.AxisListType.X

_CACHE = {}

# Prefer table sets so {Exp, Ln, Square} share one set.
_orig_tables = None


def _patched_tables(arch):
    tabs = _orig_tables(arch)
    order = ["natural_log_exp_and_others", "sigmoid_and_others"]
    out = {k: tabs[k] for k in order if k in tabs}
    out.update({k: v for k, v in tabs.items() if k not in out})
    return out


def build_program():
    return _build_program()


def _build_program():
    nc = bacc.Bacc("TRN2", target_bir_lowering=False, debug=False,
                   enable_asserts=False, num_devices=NC)

    def din(name, shape, dt):
        return nc.dram_tensor(name, shape, dt, kind="ExternalInput").ap()

    # xT_aug [9, B*G] bf16 (row 8 = ones); DMAd 4x to partition bases.
    xT_aug = din("xT_aug", [NI + 1, B * G], BF16)
    xT_loc = din("xT_loc", [NI + 1, B * GL], F32)
    # W_aug replicated at partition bases 0/32/64/96 (rows 0-8 of each 32).
    wb4 = din("wb4", [128, H], BF16)
    # folded+swizzled edge weights: [p][k][i] contiguous
    w1T = din("w1T", [128, NCH * GL], BF16)
    w2T = din("w2T", [128, NCH * GL], BF16)
    # small weights blob [34, *] bf16 (see _prep_inputs for column map)
    smallw = din("smallw", [W4G, 2 + 2 + 9 * H], F32)
    # f32 consts blob [128, *]: g_nat(4) b_nat(4) | row0: g_row(512) b_row(512) | SWe2(1)
    bnb = din("bnb", [128, 2 * LCH + 2 * GL + 1], F32)
    wrep1 = din("wrep1", [128, NCH * H], BF16)
    wrep2l = din("wrep2l", [128, LCH * H], BF16)

    out = nc.dram_tensor("out", [B * GL, NO], F32, kind="ExternalOutput").ap()
    out_r = out.rearrange("(b l p) f -> p b l f", b=B, l=LCH, p=128)

    with tile.TileContext(nc) as tc:
        with (
            tc.tile_pool(name="cp", bufs=1) as cp,
            tc.tile_pool(name="bp", bufs=1) as bp,
            tc.tile_pool(name="wp", bufs=1) as wp,
            tc.tile_pool(name="sp", bufs=8) as sp,
            tc.tile_pool(name="pp", bufs=1, space="PSUM") as pp,
            tc.tile_pool(name="dp", bufs=1, space="DRAM") as dp,
        ):
            # ---------- constant DMAs ----------
            wb4_sb = cp.tile([128, H], BF16, name="wb4_sb", tag="wb4_sb")
            nc.sync.dma_start(wb4_sb[:], wb4[:])
            smw = cp.tile([W4G, 2 + 2 + 9 * H], F32, name="smw", tag="smw")
            nc.sync.dma_start(smw[:], smallw[:])
            bnb_sb = cp.tile([128, 2 * LCH + 2 * GL + 1], F32, name="bnb_sb",
                             tag="bnb_sb")
            nc.sync.dma_start(bnb_sb[:], bnb[:])
            wrep1_sb = cp.tile([128, NCH * H], BF16, name="wrep1_sb",
                               tag="wrep1_sb")
            nc.sync.dma_start(wrep1_sb[:], wrep1[:])
            wrep2l_sb = cp.tile([128, LCH * H], BF16, name="wrep2l_sb",
                                tag="wrep2l_sb")
            nc.sync.dma_start(wrep2l_sb[:], wrep2l[:])
            xTl_sb = cp.tile([NI + 1, B * GL], F32, name="xTl_sb",
                             tag="xTl_sb")
            nc.sync.dma_start(xTl_sb[:], xT_loc[:])

            # views into the small-weight blob
            We1_dst = smw[:33, 0:1]
            We2_dst = smw[:33, 1:2]
            co = 4
            Wn1a = smw[:, co:co + H]; co += H          # [34,32]
            Wn1b = smw[:33, co:co + H]; co += H
            Wm1a = smw[:33, co:co + H]; co += H
            Wm1b = smw[:33, co:co + H]; co += H
            Wn2a = smw[:, co:co + H]; co += H          # [34,32]
            Wn2b = smw[:33, co:co + H]; co += H
            Wm2a = smw[:33, co:co + H]; co += H
            Wm2b = smw[:33, co:co + H]; co += H
            W_augf = smw[:NI + 1, co:co + H]; co += H
            bn_g_nat = bnb_sb[:, 0:LCH]
            bn_b_nat = bnb_sb[:, LCH:2 * LCH]
            bn_g_row = bnb_sb[0:1, 2 * LCH:2 * LCH + GL]
            bn_b_row = bnb_sb[0:1, 2 * LCH + GL:2 * LCH + 2 * GL]
            SWe2_col = bnb_sb[:, 2 * LCH + 2 * GL:2 * LCH + 2 * GL + 1]

            xq4 = bp.tile([NI + 1, B * G], BF16, name="xq4", tag="xq4")
            nc.sync.dma_start(xq4[:], xT_aug[:])

            # big edge-weight DMAs (contiguous per partition)
            w1T_sb = bp.tile([128, NCH * GL], BF16, name="w1T_sb", tag="w1T_sb")
            w2T_sb = bp.tile([128, NCH * GL], BF16, name="w2T_sb", tag="w2T_sb")
            QW = NCH * GL // 4
            for kq in range(4):
                nc.sync.dma_start(w1T_sb[:, kq * QW:(kq + 1) * QW],
                                  w1T[:, kq * QW:(kq + 1) * QW])
            for kq in range(4):
                nc.sync.dma_start(w2T_sb[:, kq * QW:(kq + 1) * QW],
                                  w2T[:, kq * QW:(kq + 1) * QW])

            # ---------- resident tensors / constants ----------
            ones_cb = cp.tile([1, 128], BF16, name="ones_cb", tag="ones_cb")
            nc.vector.memset(ones_cb[:], 1.0)
            ones_cf = cp.tile([1, 128], F32, name="ones_cf", tag="ones_cf")
            nc.vector.memset(ones_cf[:], 1.0)
            onesk = cp.tile([H, 1], F32, name="onesk", tag="onesk")
            nc.vector.memset(onesk[:], 1.0)

            h0n = bp.tile([128, B * NCH * W4G], BF16, name="h0n", tag="h0n")
            ghat2 = bp.tile([128, B * NCH * W4G], BF16, name="ghat2",
                            tag="ghat2")
            h0n_v = h0n.rearrange("p (q e) -> p q e", e=W4G)
            ghat2_v = ghat2.rearrange("p (q e) -> p q e", e=W4G)
            nc.vector.memset(h0n[:], 0.0)
            nc.vector.memset(ghat2[:], 0.0)
            nc.vector.memset(h0n_v[:, :, 0:1], 1.0)
            nc.vector.memset(ghat2_v[:, :, 0:1], 1.0)

            h0l = bp.tile([H + 1, B * GL], F32, name="h0l", tag="h0l")
            nodes1T = bp.tile([H + 1, B * GL], F32, name="nodes1T",
                              tag="nodes1T")
            nodes2T = bp.tile([H + 1, B * GL], F32, name="nodes2T",
                              tag="nodes2T")
            hbnT_f = bp.tile([H + 1, B * GL], F32, name="hbnT_f",
                             tag="hbnT_f")
            h1T = bp.tile([H, B * GL], F32, name="h1T", tag="h1T")
            nc.vector.memset(h0l[H:H + 1, :], 1.0)
            nc.vector.memset(nodes1T[H:H + 1, :], 1.0)
            nc.vector.memset(nodes2T[H:H + 1, :], 1.0)
            nc.vector.memset(hbnT_f[H:H + 1, :], 1.0)

            h1n = bp.tile([128, B * LCH * NO], F32, name="h1n", tag="h1n")
            h1nb = bp.tile([128, B * LCH * NO], BF16, name="h1nb", tag="h1nb")
            ghr = [bp.tile([128, NCH * NO], BF16, name=f"ghr{b}",
                           tag=f"ghr{b}") for b in range(B)]
            ssrc1 = bp.tile([128, B * NCH], F32, name="ssrc1", tag="ssrc1")
            ssrc2a = bp.tile([128, B * NCH], F32, name="ssrc2a", tag="ssrc2a")
            scl_all = bp.tile([128, NCH], F32, name="scl_all", tag="scl_all")
            shf_all = bp.tile([128, NCH], F32, name="shf_all", tag="shf_all")

            # DRAM staging for collectives
            g_in = [dp.tile([128, LCH * NO], BF16, name=f"gin{b}",
                            tag=f"gin{b}") for b in range(B)]
            g_out = [dp.tile([NC * 128, LCH * NO], BF16, addr_space="Shared",
                             name=f"gout{b}", tag=f"gout{b}")
                     for b in range(B)]
            t_in = dp.tile([128, 16], F32, name="tin", tag="tin")
            t_out = dp.tile([NC * 128, 16], F32, addr_space="Shared",
                            name="tout", tag="tout")

            def elu(z_psum, out_ap, shape, out32=None):
                """out = elu(z) = max(z, exp(min(z,0))-1)."""
                p, f = shape
                tf = wp.tile([128, GL], F32, name="elu_t", tag="elu_t",
                             bufs=3)
                t1 = tf[0:p, 0:f]
                nc.vector.tensor_scalar_min(t1, z_psum, 0.0)
                nc.scalar.activation(t1, t1, AF.Exp)
                nc.vector.tensor_scalar_add(t1, t1, -1.0)
                nc.vector.tensor_tensor(out_ap, z_psum, t1, op=ALU.max)

            # ---------- phase A: h0 natural (row-tiled) ----------
            for kq in range(8):
                ps = pp.tile([128, 8 * H], F32, name="ps_h0", tag="bc",
                             bufs=2)
                for s in range(8):
                    nc.tensor.matmul(
                        ps[:, s * H:(s + 1) * H],
                        xq4[:, kq * 1024 + s * 128:kq * 1024 + (s + 1) * 128],
                        wb4_sb[0:NI + 1, :],
                        start=True, stop=True)
                # elu into h0n groups [*, 2:34]
                tf = wp.tile([128, 8 * H], BF16, name="elu_h0", tag="elu_h0",
                             bufs=2)
                nc.vector.tensor_scalar_min(tf[:], ps[:], 0.0)
                nc.scalar.activation(tf[:], tf[:], AF.Exp)
                nc.vector.tensor_scalar_add(tf[:], tf[:], -1.0)
                tf_v = tf.rearrange("p (q f) -> p q f", f=H)
                nc.vector.tensor_tensor(
                    h0n_v[:, kq * 8:(kq + 1) * 8, 2:W4G],
                    ps.rearrange("p (q f) -> p q f", f=H), tf_v, op=ALU.max)
            # h0l transposed local (rows 0-31 = h, row 32 = ones)
            for b in range(B):
                ps = pp.tile([H, GL], F32, name="ps_h0l", tag="sm", bufs=1)
                nc.tensor.matmul(ps[:], W_augf,
                                 xTl_sb[:, b * GL:(b + 1) * GL],
                                 start=True, stop=True)
                elu(ps[:], h0l[0:H, b * GL:(b + 1) * GL], [H, GL])

            dbg_refs = {}

            # ---------- one message-passing block ----------
            def mp_block(blk, wT_sb, We_dst, Wna, Wnb, Wma, Wmb,
                         ssrc, hTl, nodesT, accum_hook, post_hook):
                """accum_hook(b, g, zb, acc) emits accum MMs or defers to
                post_hook(b, acc)."""
                sdbs = []
                accs = []
                for b in range(B):
                    # sdb: broadcast of (s_dst + b_e) row
                    ps_d = pp.tile([1, GL], F32, name="ps_d", tag="sm",
                                   bufs=1)
                    nc.tensor.matmul(ps_d[:], We_dst,
                                     hTl[:, b * GL:(b + 1) * GL],
                                     start=True, stop=True)
                    sd_row = wp.tile([1, GL], BF16, name="sd_row",
                                     tag="sd_row", bufs=2)
                    nc.vector.tensor_copy(sd_row[:], ps_d[:])
                    ps_bc = pp.tile([128, GL], F32, name="ps_bc", tag="bc",
                                    bufs=2)
                    nc.tensor.matmul(ps_bc[:], ones_cb[:], sd_row[:],
                                     start=True, stop=True)
                    sdb = wp.tile([128, GL], BF16, name="sdb", tag="sdb",
                                  bufs=2)
                    nc.vector.tensor_copy(sdb[:], ps_bc[:])
                    sdbs.append(sdb)
                for b in range(B):
                    sdb = sdbs[b]
                    ps_acc = pp.tile([W4G, GL], F32, name="ps_acc", tag="acc",
                                     bufs=2)
                    accs.append(ps_acc)
                    for g in range(NG):
                        zb = sp.tile([128, GRP * GL], BF16, name="zb",
                                     tag="zb")
                        for k4 in range(GRP):
                            k = g * GRP + k4
                            nc.vector.tensor_scalar(
                                zb[:, k4 * GL:(k4 + 1) * GL], sdb[:],
                                ssrc[:, b * NCH + k:b * NCH + k + 1], None,
                                op0=ALU.add)
                        nc.scalar.activation(zb[:], zb[:], AF.Sigmoid)
                        nc.vector.tensor_tensor(
                            zb[:], zb[:],
                            wT_sb[:, g * GRP * GL:(g + 1) * GRP * GL],
                            op=ALU.mult)
                        accum_hook(b, g, zb, ps_acc)
                    post_hook(b, ps_acc)

                # shared post-accumulation per-batch path (PE/DVE only)
                rfulls = []
                hdTs = []
                dbg_refs.setdefault(blk, {})
                for b in range(B):
                    ps_acc = accs[b]
                    rfull = wp.tile([W4G, GL], F32, name="rfull",
                                    tag="rfull", bufs=2)
                    nc.scalar.copy(rfull[:], ps_acc[:])
                    rfulls.append(rfull)
                    ps_rb = pp.tile([H, GL], F32, name="ps_rb", tag="bc",
                                    bufs=2)
                    nc.tensor.matmul(ps_rb[:], ones_cf[:, 0:H], rfull[0:1, :],
                                     start=True, stop=True)
                    hdT = wp.tile([H + 1, GL], F32, name="hdT", tag="hdT",
                                  bufs=2)
                    nc.vector.memset(hdT[H:H + 1, :], 1.0)
                    nc.vector.tensor_tensor(hdT[0:H, :],
                                            hTl[0:H, b * GL:(b + 1) * GL],
                                            ps_rb[:], op=ALU.mult)
                    hdTs.append(hdT)
                    dbg_refs[blk][f"rfull{b}"] = rfull
                    dbg_refs[blk][f"hdT{b}"] = hdT
                    ps_n = pp.tile([H, GL], F32, name="ps_n", tag="nn",
                                   bufs=3)
                    nc.tensor.matmul(ps_n[:], Wna, rfull[:],
                                     start=True, stop=False)
                    nc.tensor.matmul(ps_n[:], Wnb, hdT[:],
                                     start=False, stop=True)
                    accs[b] = (ps_acc, ps_n)

                # elu chains (ACT exp) after both batches' sigmoids
                outs = []
                for b in range(B):
                    ps_acc, ps_n = accs[b]
                    elu(ps_n[:], nodesT[0:H, b * GL:(b + 1) * GL], [H, GL])
                    ps_m = pp.tile([128, LCH * NO], F32, name="ps_m",
                                   tag="nn", bufs=3)
                    for l in range(LCH):
                        c0 = b * GL + l * 128
                        nc.tensor.matmul(ps_m[:, l * NO:(l + 1) * NO],
                                         nodesT[:, c0:c0 + 128],
                                         Wma, start=True, stop=False)
                        nc.tensor.matmul(ps_m[:, l * NO:(l + 1) * NO],
                                         hTl[:, c0:c0 + 128],
                                         Wmb, start=False, stop=True)
                    outs.append(ps_m)
                return outs

            # ---------- block 1 ----------
            # ssrc1 from natural h0 (s_src = sum_f h*We_src)
            for b in range(B):
                ssx = wp.tile([128, NCH * H], BF16, name="ssx", tag="ssx",
                              bufs=2)
                ssx_v = ssx.rearrange("p (q f) -> p q f", f=H)
                nc.vector.tensor_tensor(
                    ssx_v, h0n_v[:, b * NCH:(b + 1) * NCH, 2:W4G],
                    wrep1_sb.rearrange("p (q f) -> p q f", f=H), op=ALU.mult)
                nc.vector.reduce_sum(ssrc1[:, b * NCH:(b + 1) * NCH],
                                     ssx_v, axis=AX)

            def acc1(b, g, zb, ps_acc):
                for k4 in range(GRP):
                    k = g * GRP + k4
                    nc.tensor.matmul(ps_acc[:], h0n_v[:, b * NCH + k, :],
                                     zb[:, k4 * GL:(k4 + 1) * GL],
                                     start=(k == 0), stop=(k == NCH - 1))

            ps_ms = mp_block(1, w1T_sb, We1_dst, Wn1a, Wn1b, Wm1a, Wm1b,
                             ssrc1, h0l, nodes1T, acc1,
                             lambda b, a: None)
            for b in range(B):
                elu(ps_ms[b][:], h1n[:, b * LCH * NO:(b + 1) * LCH * NO],
                    [128, LCH * NO])
                nc.vector.tensor_copy(
                    h1nb[:, b * LCH * NO:(b + 1) * LCH * NO],
                    h1n[:, b * LCH * NO:(b + 1) * LCH * NO])
                nc.sync.dma_start(g_in[b][:],
                                  h1nb[:, b * LCH * NO:(b + 1) * LCH * NO])

            # ---------- BatchNorm stats (natural, local genes) ----------
            stat = wp.tile([128, 8 * LCH], F32, name="stat", tag="stat")
            mu_n, var_n = stat[:, 0:LCH], stat[:, LCH:2 * LCH]
            scl_n, shf_n = stat[:, 2 * LCH:3 * LCH], stat[:, 3 * LCH:4 * LCH]
            t_n = stat[:, 4 * LCH:5 * LCH]
            t2_n = stat[:, 5 * LCH:6 * LCH]
            shfSW = stat[:, 6 * LCH:7 * LCH]
            sq_n = wp.tile([128, B * LCH * NO], F32, name="sq_n", tag="sq_n")
            nc.scalar.activation(sq_n[:], h1n[:], AF.Square)
            h1n_r = h1n.rearrange("p (b l f) -> p b l f", b=B, l=LCH)
            sq_r = sq_n.rearrange("p (b l f) -> p b l f", b=B, l=LCH)
            for l in range(LCH):
                nc.vector.reduce_sum(mu_n[:, l:l + 1], h1n_r[:, :, l, :],
                                     axis=XY)
                nc.vector.reduce_sum(var_n[:, l:l + 1], sq_r[:, :, l, :],
                                     axis=XY)
            nc.vector.tensor_scalar_mul(mu_n, mu_n, 1.0 / (B * NO))
            nc.vector.tensor_scalar_mul(var_n, var_n, 1.0 / (B * NO))
            nc.vector.tensor_tensor(t_n, mu_n, mu_n, op=ALU.mult)
            nc.vector.tensor_tensor(var_n, var_n, t_n, op=ALU.subtract)
            nc.vector.tensor_scalar_add(t_n, var_n, BN_EPS)
            nc.scalar.activation(t_n, t_n, AF.Ln)
            nc.scalar.activation(t_n, t_n, AF.Exp, scale=-0.5)
            nc.vector.tensor_tensor(scl_n, t_n, bn_g_nat, op=ALU.mult)
            nc.vector.tensor_tensor(t2_n, mu_n, scl_n, op=ALU.mult)
            nc.vector.tensor_tensor(shf_n, bn_b_nat, t2_n, op=ALU.subtract)

            # ssrc2 local: scl*(sum_f h1*We2src) + shf*sum(We2src)
            red2 = wp.tile([128, B * LCH], F32, name="red2", tag="red2")
            for b in range(B):
                sx2 = wp.tile([128, LCH * NO], BF16, name="sx2", tag="sx2",
                              bufs=2)
                nc.vector.tensor_tensor(
                    sx2[:], h1n[:, b * LCH * NO:(b + 1) * LCH * NO],
                    wrep2l_sb[:], op=ALU.mult)
                nc.vector.reduce_sum(
                    red2[:, b * LCH:(b + 1) * LCH],
                    sx2.rearrange("p (l f) -> p l f", f=NO), axis=AX)
            nc.vector.tensor_scalar(shfSW, shf_n, SWe2_col, None,
                                    op0=ALU.mult)
            tpack = wp.tile([128, 16], F32, name="tpack", tag="tpack")
            for b in range(B):
                for l in range(LCH):
                    nc.vector.tensor_scalar(
                        tpack[:, b * LCH + l:b * LCH + l + 1],
                        red2[:, b * LCH + l:b * LCH + l + 1],
                        scl_n[:, l:l + 1], shfSW[:, l:l + 1],
                        op0=ALU.mult, op1=ALU.add)
            nc.vector.tensor_copy(tpack[:, 8:8 + LCH], scl_n)
            nc.vector.tensor_copy(tpack[:, 12:12 + LCH], shf_n)
            nc.sync.dma_start(t_in[:], tpack[:])

            # collectives: tiny first, then raw h1 per batch
            nc.gpsimd.collective_compute(
                "AllGather", ALU.bypass, replica_groups=[list(range(NC))],
                ins=[t_in.opt()], outs=[t_out.opt()])
            for b in range(B):
                nc.gpsimd.collective_compute(
                    "AllGather", ALU.bypass,
                    replica_groups=[list(range(NC))],
                    ins=[g_in[b].opt()], outs=[g_out[b].opt()])

            # consume tiny gather
            t_out_r = t_out.rearrange("(c p) x -> p c x", p=128)
            for b in range(B):
                nc.sync.dma_start(
                    ssrc2a[:, b * NCH:(b + 1) * NCH].rearrange(
                        "p (c l) -> p c l", c=NC),
                    t_out_r[:, :, b * LCH:(b + 1) * LCH])
            nc.sync.dma_start(
                scl_all.rearrange("p (c l) -> p c l", c=NC),
                t_out_r[:, :, 8:8 + LCH])
            nc.sync.dma_start(
                shf_all.rearrange("p (c l) -> p c l", c=NC),
                t_out_r[:, :, 12:12 + LCH])

            # ---------- BN row path -> hbnT_f, h1T ----------
            for b in range(B):
                ps = pp.tile([H, GL], F32, name="ps_h1T", tag="sm", bufs=1)
                nc.tensor.matmul(ps[:], Wm1a,
                                 nodes1T[:, b * GL:(b + 1) * GL],
                                 start=True, stop=False)
                nc.tensor.matmul(ps[:], Wm1b,
                                 h0l[:, b * GL:(b + 1) * GL],
                                 start=False, stop=True)
                elu(ps[:], h1T[:, b * GL:(b + 1) * GL], [H, GL])
            rowb = wp.tile([1, 4 * GL], F32, name="rowb", tag="rowb")
            mu_r, var_r = rowb[:, 0:GL], rowb[:, GL:2 * GL]
            scl_r, shf_r = rowb[:, 2 * GL:3 * GL], rowb[:, 3 * GL:4 * GL]
            t_r, t2_r = scl_r, shf_r
            sqT = wp.tile([H, B * GL], F32, name="sqT", tag="sqT")
            nc.scalar.activation(sqT[:], h1T[:], AF.Square)
            ps_r0 = pp.tile([1, GL], F32, name="ps_r0", tag="sm", bufs=1)
            for b in range(B):
                nc.tensor.matmul(ps_r0[:], onesk[:],
                                 h1T[:, b * GL:(b + 1) * GL],
                                 start=(b == 0), stop=(b == B - 1))
            ps_r1 = pp.tile([1, GL], F32, name="ps_r1", tag="nn", bufs=3)
            for b in range(B):
                nc.tensor.matmul(ps_r1[:], onesk[:],
                                 sqT[:, b * GL:(b + 1) * GL],
                                 start=(b == 0), stop=(b == B - 1))
            nc.vector.tensor_scalar_mul(mu_r, ps_r0[:], 1.0 / (B * NO))
            nc.vector.tensor_scalar_mul(var_r, ps_r1[:], 1.0 / (B * NO))
            nc.vector.tensor_tensor(t_r, mu_r, mu_r, op=ALU.mult)
            nc.vector.tensor_tensor(var_r, var_r, t_r, op=ALU.subtract)
            nc.vector.tensor_scalar_add(t_r, var_r, BN_EPS)
            nc.scalar.activation(t_r, t_r, AF.Ln)
            nc.scalar.activation(t_r, t_r, AF.Exp, scale=-0.5)
            nc.vector.tensor_tensor(scl_r, t_r, bn_g_row, op=ALU.mult)
            nc.vector.tensor_tensor(t2_r, mu_r, scl_r, op=ALU.mult)
            nc.vector.tensor_tensor(shf_r, bn_b_row, t2_r, op=ALU.subtract)
            ps_sc = pp.tile([H, GL], F32, name="ps_sc", tag="bc", bufs=2)
            nc.tensor.matmul(ps_sc[:], ones_cf[:, 0:H], scl_r,
                             start=True, stop=True)
            ps_sh = pp.tile([H, GL], F32, name="ps_sh", tag="bc", bufs=2)
            nc.tensor.matmul(ps_sh[:], ones_cf[:, 0:H], shf_r,
                             start=True, stop=True)
            for b in range(B):
                sl = slice(b * GL, (b + 1) * GL)
                nc.vector.tensor_tensor(hbnT_f[0:H, sl], h1T[:, sl],
                                        ps_sc[:], op=ALU.mult)
                nc.vector.tensor_tensor(hbnT_f[0:H, sl], hbnT_f[0:H, sl],
                                        ps_sh[:], op=ALU.add)

            # ---------- block 2 (overlapped with raw gathers) ----------
            def norm_batch(b):
                """Fill ghat2 groups for batch b from the raw gather."""
                gr = ghr[b]
                go_r = g_out[b].rearrange("(c p) x -> p c x", p=128)
                nc.sync.dma_start(gr.rearrange("p (c l f) -> p c (l f)",
                                               c=NC, l=LCH),
                                  go_r[:, :, :])
                # shf column for all chunks of this batch
                nc.vector.tensor_copy(
                    ghat2_v[:, b * NCH:(b + 1) * NCH, 1:2],
                    shf_all.rearrange("p (q o) -> p q o", o=1))
                for q in range(NCH):
                    nc.vector.tensor_scalar(
                        ghat2_v[:, b * NCH + q, 2:W4G],
                        gr[:, q * NO:(q + 1) * NO],
                        scl_all[:, q:q + 1], None, op0=ALU.mult)

            zb_saved = {}

            def acc2_save(b, g, zb, ps_acc):
                zb_saved[(b, g)] = zb

            def post2(b, ps_acc):
                norm_batch(b)
                for g in range(NG):
                    zb = zb_saved[(b, g)]
                    for k4 in range(GRP):
                        k = g * GRP + k4
                        nc.tensor.matmul(ps_acc[:],
                                         ghat2_v[:, b * NCH + k, :],
                                         zb[:, k4 * GL:(k4 + 1) * GL],
                                         start=(k == 0), stop=(k == NCH - 1))

            ps_ms2 = mp_block(2, w2T_sb, We2_dst, Wn2a, Wn2b, Wm2a, Wm2b,
                              ssrc2a, hbnT_f, nodes2T, acc2_save,
                              post2)
            out_n = wp.tile([128, B * LCH * NO], F32, name="out_n",
                            tag="out_n")
            for b in range(B):
                elu(ps_ms2[b][:], out_n[:, b * LCH * NO:(b + 1) * LCH * NO],
                    [128, LCH * NO])
            nc.sync.dma_start(out_r, out_n[:])
            import os as _os
            if _os.environ.get("DBG_DUMP", "0") == "1":
                for nm, t in [("dbg_w1T", w1T_sb[:, 0:2048]),
                              ("dbg_w1Tb", w1T_sb[:, 14336:16384]),
                              ("dbg_rfull1", dbg_refs[1]["rfull0"][:]),
                              ("dbg_hdT1", dbg_refs[1]["hdT0"][:]),
                              ("dbg_h0n", h0n[:, 0:512]),
                              ("dbg_h0l", h0l[:, 0:512]),
                              ("dbg_ssrc1", ssrc1[:, :]),
                              ("dbg_nodes1", nodes1T[:, 0:512]),
                              ("dbg_h1n", h1n[:, :]),
                              ("dbg_hbnT", hbnT_f[:, 0:512]),
                              ("dbg_ghat2", ghat2[:, 0:512]),
                              ("dbg_ssrc2", ssrc2a[:, :]),
                              ("dbg_sq", stat[:, :])]:
                    dt_ = t.dtype
                    dto = nc.dram_tensor(nm, list(t.shape), dt_,
                                         kind="ExternalOutput").ap()
                    nc.sync.dma_start(dto, t)

    nc.compile()
    return nc


def _prep_inputs(x, edges1, edges2, W_infer, b_infer, W_e1, b_e1, W_e2, b_e2,
                 W_n1, b_n1, W_n2, b_n2, W_m1, b_m1, W_m2, b_m2,
                 bn_gamma, bn_beta):
    f32 = np.float32
    bf16 = ml_dtypes.bfloat16
    xT = np.asarray(x, f32).transpose(2, 0, 1).reshape(NI, B * G)
    xT_aug = np.concatenate([xT, np.ones((1, B * G), f32)], 0).astype(bf16)
    w1 = (ALPHA + (1.0 - ALPHA) * np.asarray(edges1, f32)).astype(bf16)
    w2 = (BETA + (1.0 - BETA) * np.asarray(edges2, f32)).astype(bf16)

    def swz(w):
        # [j, i_local] -> [p][k][i] contiguous per partition
        wt = np.ascontiguousarray(w.T)  # [G(j), GL]
        return np.ascontiguousarray(
            wt.reshape(NCH, 128, GL).transpose(1, 0, 2).reshape(
                128, NCH * GL))

    # W_aug replicated at partition bases 0/32/64/96
    W_aug = np.concatenate([np.asarray(W_infer, f32),
                            np.asarray(b_infer, f32)[None, :]], 0)
    wb4 = np.zeros((128, H), f32)
    for r in range(4):
        wb4[32 * r:32 * r + NI + 1] = W_aug
    wb4 = wb4.astype(bf16)

    z32 = np.zeros((1, NO), f32)

    def rows34(*rs):
        m = np.concatenate(rs, 0)
        assert m.shape[0] <= W4G
        if m.shape[0] < W4G:
            m = np.concatenate([m, np.zeros((W4G - m.shape[0], m.shape[1]),
                                            f32)], 0)
        return m

    W_n1_, W_n2_ = np.asarray(W_n1, f32), np.asarray(W_n2, f32)
    W_m1_, W_m2_ = np.asarray(W_m1, f32), np.asarray(W_m2, f32)
    cols = []
    # We1_dst / We2_dst: [We[H:,0]; b_e] padded to 34 rows
    cols.append(rows34(np.asarray(W_e1, f32)[H:, 0:1],
                       np.asarray(b_e1, f32)[None, :]))
    cols.append(rows34(np.asarray(W_e2, f32)[H:, 0:1],
                       np.asarray(b_e2, f32)[None, :]))
    cols.append(np.zeros((W4G, 2), f32))  # pad to col 4
    cols.append(rows34(np.zeros((2, NO), f32), W_n1_[:H]))          # Wn1a
    cols.append(rows34(W_n1_[H:], np.asarray(b_n1, f32)[None, :]))  # Wn1b
    cols.append(rows34(W_m1_[:H], np.asarray(b_m1, f32)[None, :]))  # Wm1a
    cols.append(rows34(W_m1_[H:], z32))                             # Wm1b
    cols.append(rows34(z32, np.sum(W_n2_[:H], 0)[None, :],
                       W_n2_[:H]))                                  # Wn2a
    cols.append(rows34(W_n2_[H:], np.asarray(b_n2, f32)[None, :]))  # Wn2b
    cols.append(rows34(W_m2_[:H], np.asarray(b_m2, f32)[None, :]))  # Wm2a
    cols.append(rows34(W_m2_[H:], z32))                             # Wm2b
    cols.append(rows34(np.asarray(W_infer, f32),
                       np.asarray(b_infer, f32)[None, :]))          # W_augf
    smallw = np.concatenate(cols, 1).astype(f32)

    wrep1 = np.tile(np.asarray(W_e1, f32)[:H, 0], NCH)[None, :].repeat(
        128, 0).astype(bf16)
    wrep2l = np.tile(np.asarray(W_e2, f32)[:H, 0], LCH)[None, :].repeat(
        128, 0).astype(bf16)
    SWe2 = float(np.asarray(W_e2, f32)[:H, 0].sum())

    in_maps = []
    for c in range(NC):
        sl = slice(c * GL, (c + 1) * GL)
        xl = np.asarray(x, f32)[:, sl, :].transpose(2, 0, 1).reshape(
            NI, B * GL)
        m = dict(xT_aug=xT_aug, wb4=wb4, smallw=smallw, wrep1=wrep1,
                 wrep2l=wrep2l)
        m["xT_loc"] = np.concatenate(
            [xl, np.ones((1, B * GL), f32)], 0)
        m["w1T"] = swz(w1[sl, :])
        m["w2T"] = swz(w2[sl, :])
        g = np.asarray(bn_gamma, f32)[sl]
        b_ = np.asarray(bn_beta, f32)[sl]
        bnb = np.zeros((128, 2 * LCH + 2 * GL + 1), f32)
        bnb[:, 0:LCH] = g.reshape(LCH, 128).T
        bnb[:, LCH:2 * LCH] = b_.reshape(LCH, 128).T
        bnb[0, 2 * LCH:2 * LCH + GL] = g
        bnb[0, 2 * LCH + GL:2 * LCH + 2 * GL] = b_
        bnb[:, 2 * LCH + 2 * GL] = SWe2
        m["bnb"] = bnb
        in_maps.append(m)
    return in_maps


def kernel(**inputs):
    if "nc" not in _CACHE:
        _CACHE["nc"] = build_program()
    nc = _CACHE["nc"]
    in_maps = _prep_inputs(**inputs)
    res = run_bass_kernel_spmd(nc, in_maps, list(range(NC)))
    parts = [res.results[c]["out"].reshape(B, GL, NO) for c in range(NC)]
    return np.concatenate(parts, axis=1).astype(np.float32)
